# revision 35
# baseline (speedup 1.0000x reference)
"""Multi-head attention kernel for 8 Trainium2 NeuronCores.

Problem: x[4,2048,768] -> qkv proj (w_qkv[768,2304]) -> 12-head attention
(head_dim 64) -> out proj (w_proj[768,768]).

Sharding: 8 cores, each handles one (batch, head-group-of-6) pair:
core c -> batch c//2, heads (c%2)*6 .. +6. Each core computes its 6 heads'
qkv projections, attention, and the partial output projection
sum_h attnout_h @ w_proj[h-rows]. Host sums the two half-head partials per
batch. No inter-core communication needed.

On-device layout (all matmul operands float32r -> 1 cycle/row on PE):
  xT   [768, 2048]   x[b] transposed (host-prepared)
  qT/kT [384, 2048]  features on partitions (3 blocks of 128 = 6 heads)
  v    16 tiles [128, 768]: per head j 128-col slot: even j [v_j|ones],
       odd j [ones|v_j] (the ones columns produce softmax denominators
       replicated over 64 partitions in the same matmul as attn@V)
  Attention is computed fully transposed: S^T = K Q^T per 128-key block,
  P^T = exp(S^T / 8) via ACT (psum -> sbuf f32r), out^T accumulates
  lhsT=[v|ones] over key blocks. Softmax division happens on out^T via
  fast reciprocal + an SBUF->SBUF DMA partition shift + one DVE multiply.

PSUM: one pool, four 2-bank tags (sA, sB, poA, poB) shared by all phases so
no pool-boundary serialization exists; per-tile FIFO deps give pipelining.
"""

import ml_dtypes
import numpy as np

import concourse.bass as bass
import concourse.mybir as mybir
import concourse.tile as tile
from concourse import bacc
from concourse.bass_utils import run_bass_kernel_spmd

F32 = mybir.dt.float32
F32R = mybir.dt.float32r
BF16 = mybir.dt.bfloat16
PV_BF16 = True   # bf16 P/V: required so the DVE fast-exp bit trick (int16
                 # convert-on-write, bits read as bf16) passes walrus's
                 # "f32r matmult inputs must be f32r-rounded" verifier rule;
                 # bf16 P/V alone costs ~2e-3 rel err (gate 2e-2)
PV_DT = BF16 if PV_BF16 else F32R
EXP = mybir.ActivationFunctionType.Exp
MULT = mybir.AluOpType.mult

B, N, DIM, HEADS, HD = 4, 2048, 768, 12, 64
NH = 6                 # heads per core
NPAIR = NH // 2        # head pairs per core
FQ = NH * HD           # 384 per-core q/k/v feature count
DC = DIM // 128        # 6 contraction chunks
TB = N // 128          # 16 token blocks
QH = 4                 # query quarters
QHW = N // QH          # 512
SCALE = HD ** -0.5

# Softmax-exp engine split: ACT (hardware exp, 1 elem/cyc @1.2GHz) handles
# most key-blocks; DVE handles DVE_KBS via the bf16 Schraudolph bit trick
# exp(y) ~= bitcast_bf16(int16(A*y + B)) done as ONE tensor_scalar
# (mult+add, int16 convert-on-write), ~3% sawtooth rel err on those blocks
# only; numerator and denominator use the same approx so softmax stays
# normalized.  End-to-end rel err measured 9.0e-3 at 6/16 blocks (gate 2e-2).
DVE_KBS = (2, 5, 8, 11, 13, 15)
FEXP_A = (1 << 7) / float(np.log(2.0))    # 2^7 * log2(e) = 184.664
FEXP_B = 127.0 * (1 << 7) - 7.0           # bias minus sawtooth-centering C

_cache = {}


def _build(repeats=1):
    nc = bacc.Bacc("TRN2", target_bir_lowering=False, debug=False)

    xT_d = nc.dram_tensor("xT", [DIM, N], F32R, kind="ExternalInput")
    wq_d = nc.dram_tensor("wq", [DIM, FQ], F32R, kind="ExternalInput")
    wk_d = nc.dram_tensor("wk", [DIM, FQ], F32R, kind="ExternalInput")
    wv_d = nc.dram_tensor("wv", [DIM, FQ], F32R, kind="ExternalInput")
    bq_d = nc.dram_tensor("bq", [128, NPAIR], F32, kind="ExternalInput")
    bk_d = nc.dram_tensor("bk", [128, NPAIR], F32, kind="ExternalInput")
    wp_d = nc.dram_tensor("wp", [FQ, DIM], F32R, kind="ExternalInput")
    ones_d = nc.dram_tensor("ones", [128, NPAIR * HD], PV_DT, kind="ExternalInput")
    out_d = nc.dram_tensor("out", [N, DIM], F32, kind="ExternalOutput")

    with tile.TileContext(nc) as tc:
        with (
            tc.tile_pool(name="persist", bufs=1) as persist,
            tc.tile_pool(name="ps", bufs=1, space="PSUM") as ps_pool,
        ):
            # cross-phase tiles: wp 9KB + qT/kT 48KB + v 48KB = 105KB/part
            wp = []
            for p in range(NPAIR):
                t = persist.tile([128, DIM], F32R, tag=f"wp{p}", name=f"wp{p}")
                nc.sync.dma_start(t[:], wp_d[p * 128:(p + 1) * 128, :])
                wp.append(t)
            qT = [persist.tile([128, N], F32R, tag=f"qT{p}", name=f"qT{p}") for p in range(NPAIR)]
            kT = [persist.tile([128, N], F32R, tag=f"kT{p}", name=f"kT{p}") for p in range(NPAIR)]
            v_sb = [persist.tile([128, 2 * FQ], PV_DT, tag=f"v{tb}", name=f"v{tb}") for tb in range(TB)]

            ps_tags = ["sA", "sB", "poA", "poB"]

            def ps_tile(i, width=QHW):
                return ps_pool.tile([128, width], F32, tag=ps_tags[i % 4], name=f"ps_{ps_tags[i % 4]}")

            for _rep in range(repeats):
              # =============== phase 1: QKV projections ======================
              with tc.tile_pool(name="qkv_in", bufs=1) as qkv_in:
                  bq = qkv_in.tile([128, NPAIR], F32, tag="bq")
                  nc.sync.dma_start(bq[:], bq_d[:])
                  bk = qkv_in.tile([128, NPAIR], F32, tag="bk")
                  nc.sync.dma_start(bk[:], bk_d[:])
                  # interleave x/weight chunk DMAs so dc-0 operands land first
                  xT, wq, wk, wv = [], [], [], []
                  for dc in range(DC):
                      t = qkv_in.tile([128, N], F32R, tag=f"xT{dc}", name=f"xT{dc}")
                      nc.sync.dma_start(t[:], xT_d[dc * 128:(dc + 1) * 128, :])
                      xT.append(t)
                      for (lst, src, nm) in ((wv, wv_d, "wv"), (wq, wq_d, "wq"), (wk, wk_d, "wk")):
                          t = qkv_in.tile([128, FQ], F32R, tag=f"{nm}{dc}", name=f"{nm}{dc}")
                          nc.sync.dma_start(t[:], src[dc * 128:(dc + 1) * 128, :])
                          lst.append(t)

                  # V first so attention's S pipeline frees up early
                  ones_fills = []
                  for tb in range(TB):
                      ps = ps_pool.tile([128, FQ], F32, tag=["poA", "poB", "sC", "sA", "sB"][tb % 5], name="ps_v")
                      for dc in range(DC):
                          nc.tensor.matmul(
                              ps[:],
                              xT[dc][:, tb * 128:(tb + 1) * 128],
                              wv[dc][:],
                              start=(dc == 0), stop=(dc == DC - 1),
                          )
                      t = v_sb[tb]
                      t4 = t[:].rearrange("p (g s c) -> p g s c", g=NPAIR, s=4, c=HD)
                      ps4 = ps[:].rearrange("p (g s c) -> p g s c", g=NPAIR, s=2, c=HD)
                      ones3 = ones_d[:].rearrange("p (g c) -> p g c", g=NPAIR, c=HD)
                      # ACT copies: keeps DVE free; ACT is idle in phase 1
                      nc.scalar.copy(t4[:, :, 0, :], ps4[:, :, 0, :])  # even-head v
                      nc.scalar.copy(t4[:, :, 3, :], ps4[:, :, 1, :])  # odd-head v
                      ones_fills.append(t4)

                  # q/k units: [128, 512] accumulations; fb0 before attention
                  # (rotating 3 free tags), fb1/fb2 emitted inside the attention
                  # stream (qkpj tag) so the scheduler hides them in PE slack
                  def qk_unit(fb, qi, h4, tag):
                      dst, w_sb, b_sb = (qT, wq, bq) if qi == 0 else (kT, wk, bk)
                      ps = ps_pool.tile([128, QHW], F32, tag=tag, name="ps_qk")
                      for dc in range(DC):
                          nc.tensor.matmul(
                              ps[:],
                              w_sb[dc][:, fb * 128:(fb + 1) * 128],
                              xT[dc][:, h4 * QHW:(h4 + 1) * QHW],
                              start=(dc == 0), stop=(dc == DC - 1),
                          )
                      nc.scalar.add(
                          dst[fb][:, h4 * QHW:(h4 + 1) * QHW], ps[:], b_sb[:, fb:fb + 1])

                  it = 0
                  for fb in range(NPAIR):
                      for qi in range(2):
                          for h4 in range(4):
                              qk_unit(fb, qi, h4, ["poA", "poB", "sC", "sA", "sB"][it % 5])
                              it += 1
                  # ones fills deferred here: 3MB of constant DMA traffic kept out
                  # of the prologue's critical xT/weight streaming window (the
                  # ones columns are first read by attn@V, well after this point)
                  for t4 in ones_fills:
                      nc.sync.dma_start(t4[:, :, 1, :], ones3)
                      nc.sync.dma_start(t4[:, :, 2, :], ones3)

            # ================= phase 2: attention ============================
              with (
                  tc.tile_pool(name="attn_sb", bufs=1) as attn_sb,
                  tc.tile_pool(name="p_pool", bufs=10) as p_pool,
                  tc.tile_pool(name="rec_pool", bufs=4) as rec_pool,
                  tc.tile_pool(name="ot_pool", bufs=3) as ot_pool,
              ):
                  attnT = [attn_sb.tile([128, N], F32R, tag=f"attnT{p}", name=f"attnT{p}") for p in range(NPAIR)]

                  def proj_piece(tb_lo, tb_hi):
                      for tb in range(tb_lo, tb_hi):
                          ps = ps_pool.tile([128, DIM], F32, tag=["sA", "sB", "sC"][tb % 3], name="ps_pj")
                          for p in range(NPAIR):
                              for (lo, hi) in ((0, 512), (512, DIM)):
                                  nc.tensor.matmul(
                                      ps[:, lo:hi],
                                      attnT[p][:, tb * 128:(tb + 1) * 128],
                                      wp[p][:, lo:hi],
                                      start=(p == 0), stop=(p == NPAIR - 1),
                                  )
                          ot = ot_pool.tile([128, DIM], F32, tag="ot")
                          # alternate drain engine so neither ACT nor DVE
                          # becomes the phase-2 straggler
                          if tb % 2 == 0:
                              nc.scalar.copy(ot[:], ps[:])
                          else:
                              nc.vector.tensor_copy(ot[:], ps[:])
                          nc.sync.dma_start(out_d[tb * 128:(tb + 1) * 128, :], ot[:])

                  for qq in range(QH):
                      qs = slice(qq * QHW, (qq + 1) * QHW)
                      for p in range(NPAIR):
                          poA = ps_pool.tile([128, QHW], F32, tag="poA", name="poA")
                          poB = ps_pool.tile([128, QHW], F32, tag="poB", name="poB")

                          # Software-pipelined: the PE's NX dispatches strictly
                          # in order, so a PV matmul emitted right after its exp
                          # serializes the whole stream on the exp latency
                          # (S 0.2us -> exp 1.0us -> PV 0.4us per kb).  Pairwise
                          # emission (S(2m), S(2m+1), PV(2m-2), PV(2m-1)) puts
                          # ~1.5us of dispatchable PE work between an exp issue
                          # and the first matmul that waits on it.
                          pqueue = []

                          def s_exp(kb):
                              ks = slice(kb * 128, (kb + 1) * 128)
                              # fused-pair S tile: cols 0:512 head A, 512:1024 head B.
                              # 3-deep rotation: exp(kb) no longer gates S(kb+2),
                              # so the kb pipeline is engine-throughput paced.
                              sAB = ps_pool.tile([128, 2 * QHW], F32, tag=["sA", "sB", "sC"][kb % 3], name="sAB")
                              nc.tensor.matmul(sAB[:, 0:QHW], kT[p][0:64, ks], qT[p][0:64, qs],
                                               start=True, stop=True)
                              nc.tensor.matmul(sAB[:, QHW:2 * QHW], kT[p][64:128, ks], qT[p][64:128, qs],
                                               start=True, stop=True)
                              pAB = p_pool.tile([128, 2 * QHW], PV_DT, tag="pt", name="pAB")
                              if kb in DVE_KBS:
                                  # fast-exp bit trick on DVE: int16(A*SCALE*s + B)
                                  # then the PV matmul reads the bits as bf16
                                  nc.vector.tensor_scalar(
                                      pAB[:].bitcast(mybir.dt.int16), sAB[:],
                                      FEXP_A * SCALE, FEXP_B, MULT, mybir.AluOpType.add)
                              else:
                                  nc.scalar.activation(pAB[:], sAB[:], EXP, scale=SCALE)
                              pqueue.append((kb, pAB))

                          def pv(kb, pAB):
                              nc.tensor.matmul(poA[:], v_sb[kb][:, (2 * p) * 128:(2 * p + 1) * 128],
                                               pAB[:, 0:QHW], start=(kb == 0), stop=(kb == TB - 1))
                              nc.tensor.matmul(poB[:], v_sb[kb][:, (2 * p + 1) * 128:(2 * p + 2) * 128],
                                               pAB[:, QHW:2 * QHW], start=(kb == 0), stop=(kb == TB - 1))

                          for m in range(TB // 2):
                              s_exp(2 * m)
                              s_exp(2 * m + 1)
                              if m >= 1:
                                  pv(*pqueue.pop(0))
                                  pv(*pqueue.pop(0))
                          pv(*pqueue.pop(0))
                          pv(*pqueue.pop(0))
                          # softmax division: full-tile fast recip (custom op breaks
                          # on partition-offset APs), partition-shift DMA, multiply
                          recA = rec_pool.tile([128, QHW], F32, tag="rec", name="recA")
                          nc.vector.reciprocal_approx_fast(recA[:], poA[:])
                          nc.sync.dma_start(recA[0:64, :], recA[64:128, :])
                          nc.vector.tensor_tensor(attnT[p][0:64, qs], poA[0:64, :], recA[0:64, :], MULT)
                          recB = rec_pool.tile([128, QHW], F32, tag="rec", name="recB")
                          nc.vector.reciprocal_approx_fast(recB[:], poB[:])
                          nc.sync.dma_start(recB[64:128, :], recB[0:64, :])
                          nc.vector.tensor_tensor(attnT[p][64:128, qs], poB[64:128, :], recB[64:128, :], MULT)

                  # phase 3: output projection.  Runs after attention so the S
                  # tags could be 3-deep during phase 2 (the old in-stream qkpj
                  # tag's banks went to sC); proj psum reuses the S tags.
                  proj_piece(0, 16)

    nc.compile()
    return nc


def _get_nc():
    if "nc" not in _cache:
        _cache["nc"] = _build()
    return _cache["nc"]


def make_in_maps(x, w_qkv, b_qkv, w_proj, b_proj):
    x = np.asarray(x, dtype=np.float32)
    w_qkv = np.asarray(w_qkv, dtype=np.float32)
    b_qkv = np.asarray(b_qkv, dtype=np.float32)
    w_proj = np.asarray(w_proj, dtype=np.float32)
    ones = np.ones((128, NPAIR * HD), dtype=np.float32)
    in_maps = []
    for c in range(8):
        b = c // 2
        h0 = (c % 2) * NH
        f0 = h0 * HD
        in_maps.append({
            "xT": np.ascontiguousarray(x[b].T),
            "wq": np.ascontiguousarray(w_qkv[:, f0:f0 + FQ]),
            "wk": np.ascontiguousarray(w_qkv[:, DIM + f0:DIM + f0 + FQ]),
            "wv": np.ascontiguousarray(w_qkv[:, 2 * DIM + f0:2 * DIM + f0 + FQ]),
            "bq": np.ascontiguousarray(b_qkv[f0:f0 + FQ].reshape(NPAIR, 128).T),
            "bk": np.ascontiguousarray(b_qkv[DIM + f0:DIM + f0 + FQ].reshape(NPAIR, 128).T),
            "wp": np.ascontiguousarray(w_proj[f0:f0 + FQ, :]),
            "ones": ones if not PV_BF16 else ones.astype(ml_dtypes.bfloat16),
        })
    return in_maps


def combine(results, b_qkv, b_proj, w_proj):
    b_qkv = np.asarray(b_qkv, dtype=np.float32)
    b_proj = np.asarray(b_proj, dtype=np.float32)
    w_proj = np.asarray(w_proj, dtype=np.float32)
    # exact v-bias correction: attnout_h gains +bv_h, so out gains bv @ w_proj
    bias = b_proj + b_qkv[2 * DIM:] @ w_proj
    out = np.empty((B, N, DIM), dtype=np.float32)
    for b in range(B):
        out[b] = results[2 * b]["out"] + results[2 * b + 1]["out"] + bias
    return out


def kernel(x, w_qkv, b_qkv, w_proj, b_proj):
    nc = _get_nc()
    in_maps = make_in_maps(x, w_qkv, b_qkv, w_proj, b_proj)
    res = run_bass_kernel_spmd(nc, in_maps, core_ids=list(range(8)))
    return combine(res.results, b_qkv, b_proj, w_proj)

